# revision 2
# baseline (speedup 1.0000x reference)
"""Multi-head self-attention (ANE-style 1x1-conv attention) on 8 trn2 cores.

Sharding: (batch, head-group) tensor parallel. Core c handles batch
b = c//2 and heads [8*(c%2), 8*(c%2)+8) over the FULL sequence, i.e.
the q/k/v projection weights are split 512 out-features per core and
the out-projection contraction is split 512 in-features per core. The
two cores of a batch return partial yT sums; the host adds them (plus
the bias) during the gather -- no on-device collective and no
duplicated k/v projection work.

Per-core pipeline (fp16 operands, fp32 PSUM accumulation):
  1. q/k proj: out (o, l) -> resident SBUF per head-pair (128 = 2*Dh
     partitions, L free).
  2. v proj emitted pre-transposed (lhsT = x chunk, rhs = wvT) so V
     lands as (l on partitions, features free) -- straight into the
     AV stationary layout, no DRAM spill. A ones column is appended
     per head (vaug 65-wide) so the AV matmul also accumulates the
     softmax denominator in PSUM row 64.
  3. attention per (pair, i-quarter of 512): scores are TWO K=64
     row-tiled matmuls (tile_position (0,0)/(64,0)) running
     concurrently on the PE array -- head-even/odd share the array
     instead of zero-padding K to 128. One exp ACT per jc covers both
     heads (128 x 1024 PSUM tile). AV matmuls (M=65) trail the exp
     stream by 2 jc. ACT (1 elem/lane/cycle) is the pacing engine;
     projection / out-projection psum-groups are injected as PE
     fillers inside the jc loops.
  4. normalize off the PE (denominator row -> DRAM -> 128-lane
     reciprocal -> DRAM -> partition-broadcast -> DVE multiply).
  5. out-proj per i-quarter as fillers/tail: yT partial (fp16) to
     DRAM; host sums core pairs and adds the bias.
"""

import numpy as np

import concourse.bass as bass
import concourse.tile as tile
from concourse import bacc, mybir
from concourse.bass_utils import run_bass_kernel_spmd

B, D, L, H, Dh = 4, 1024, 2048, 16, 64
NCORES = 8
F32 = mybir.dt.float32
F16 = mybir.dt.float16
ACT_EXP = mybir.ActivationFunctionType.Exp
INV_SCALE = 1.0 / 8.0  # 1/sqrt(Dh)

NP = D // 128      # 8 x-chunks of the model dim
NPAIR = 4          # head pairs per core (8 heads)
NJC = L // 128     # 16 key chunks
NIQ = 4            # query quarters
IQ = L // NIQ      # 512 queries per quarter
NO = 512           # projected features per core (8 heads * 64)


def build_nc():
    nc = bacc.Bacc()
    x = nc.dram_tensor("x", [D, L], F16, kind="ExternalInput")
    wqT = nc.dram_tensor("wqT", [D, NO], F16, kind="ExternalInput")
    wkT = nc.dram_tensor("wkT", [D, NO], F16, kind="ExternalInput")
    wvT = nc.dram_tensor("wvT", [D, NO], F16, kind="ExternalInput")
    woT = nc.dram_tensor("woT", [NO, D], F16, kind="ExternalInput")
    ones16 = nc.dram_tensor("ones16", [1, NJC], F16, kind="ExternalInput")
    yT = nc.dram_tensor("yT", [L, D], F16, kind="ExternalOutput")

    with tile.TileContext(nc) as tc:
        with (
            nc.allow_low_precision(reason="fp16 operands by design"),
            tc.tile_pool(name="dram", bufs=1, space="DRAM") as dram,
            tc.tile_pool(name="keep", bufs=1) as keep,
            tc.tile_pool(name="attn", bufs=1) as attn,
            tc.tile_pool(name="ps", bufs=2, space="PSUM") as ps,
        ):
            # ---------------- resident SBUF tensors ----------------
            x_sb, wq_sb, wk_sb, wv_sb = [], [], [], []
            for kc in range(NP):
                xt = keep.tile([128, L], F16, name=f"x{kc}", tag=f"x{kc}")
                eng = nc.sync if kc % 2 == 0 else nc.gpsimd
                eng.dma_start(out=xt, in_=x[128 * kc:128 * (kc + 1), :])
                x_sb.append(xt)
            for kc in range(NP):
                wt = keep.tile([128, NO], F16, name=f"wq{kc}", tag=f"wq{kc}")
                nc.sync.dma_start(out=wt, in_=wqT[128 * kc:128 * (kc + 1), :])
                wq_sb.append(wt)
            for kc in range(NP):
                wt = keep.tile([128, NO], F16, name=f"wk{kc}", tag=f"wk{kc}")
                nc.sync.dma_start(out=wt, in_=wkT[128 * kc:128 * (kc + 1), :])
                wk_sb.append(wt)
            for kc in range(NP):
                wt = keep.tile([128, NO], F16, name=f"wv{kc}", tag=f"wv{kc}")
                nc.gpsimd.dma_start(out=wt, in_=wvT[128 * kc:128 * (kc + 1), :])
                wv_sb.append(wt)
            wo_sb = []
            for kc in range(NPAIR):
                wt = keep.tile([128, D], F16, name=f"wo{kc}", tag=f"wo{kc}")
                nc.gpsimd.dma_start(out=wt, in_=woT[128 * kc:128 * (kc + 1), :])
                wo_sb.append(wt)

            q_res = [attn.tile([128, L], F16, name=f"q{t}", tag=f"q{t}")
                     for t in range(NPAIR)]
            k_res = [attn.tile([128, L], F16, name=f"k{t}", tag=f"k{t}")
                     for t in range(NPAIR)]
            o_res = [attn.tile([128, L], F16, name=f"o{t}", tag=f"o{t}")
                     for t in range(NPAIR)]
            # vaug: (j-part, jc, [V_he(64) | 1 | V_ho(64) | 1]) per pair
            vaug = [attn.tile([128, NJC, 130], F16, name=f"v{t}",
                              tag=f"v{t}") for t in range(NPAIR)]
            for t in range(NPAIR):
                for e in range(2):
                    nc.gpsimd.dma_start(
                        out=vaug[t][:, :, 65 * e + 64:65 * e + 65],
                        in_=bass.AP(tensor=ones16, offset=0,
                                    ap=[[0, 128], [1, NJC], [1, 1]]))

            def ps_s():
                return ps.tile([128, 1024], F32, name="ps_s", tag="ps_s",
                               bufs=2)

            def ps_o():
                return ps.tile([128, IQ], F32, name="ps_o", tag="ps_o",
                               bufs=2)

            def ps_f():
                return ps.tile([128, 512], F32, name="ps_f", tag="ps_f",
                               bufs=2)

            # ---------------- projection psum-group units ----------------
            def emit_qk_unit(t, n, which):
                w_sb = wq_sb if which == "q" else wk_sb
                dst = q_res[t] if which == "q" else k_res[t]
                g_ps = ps_f()
                for kc in range(NP):
                    nc.tensor.matmul(
                        g_ps, lhsT=w_sb[kc][:, 128 * t:128 * (t + 1)],
                        rhs=x_sb[kc][:, 512 * n:512 * (n + 1)],
                        start=(kc == 0), stop=(kc == NP - 1))
                nc.vector.tensor_copy(out=dst[:, 512 * n:512 * (n + 1)],
                                      in_=g_ps)

            def emit_v_unit(lc):
                # vT chunk: (128 l-partitions, 512 features), lhsT = x chunk
                g_ps = ps_f()
                for kc in range(NP):
                    nc.tensor.matmul(
                        g_ps, lhsT=x_sb[kc][:, 128 * lc:128 * (lc + 1)],
                        rhs=wv_sb[kc], start=(kc == 0), stop=(kc == NP - 1))
                for t in range(NPAIR):
                    src = g_ps[:, 128 * t:128 * (t + 1)].rearrange(
                        "p (e c) -> p e c", e=2)
                    dst = vaug[t][:, lc].rearrange(
                        "p (e c) -> p e c", c=65)[:, :, 0:64]
                    nc.vector.tensor_copy(out=dst, in_=src)

            def emit_o_unit(iq, mi, n):
                g_ps = ps_f()
                for kc in range(NPAIR):
                    nc.tensor.matmul(
                        g_ps,
                        lhsT=o_res[kc][:, 512 * iq + 128 * mi:
                                       512 * iq + 128 * (mi + 1)],
                        rhs=wo_sb[kc][:, 512 * n:512 * (n + 1)],
                        start=(kc == 0), stop=(kc == NPAIR - 1))
                ysb = attn.tile([128, 512], F16, name="ysb", tag="ysb",
                                bufs=4)
                nc.vector.tensor_copy(out=ysb, in_=g_ps)
                nc.gpsimd.dma_start(
                    out=yT[512 * iq + 128 * mi:512 * iq + 128 * (mi + 1),
                           512 * n:512 * (n + 1)],
                    in_=ysb)

            # ---------------- attention ----------------
            def emit_attn(t, iq, fillers=()):
                fillers = list(fillers)
                o_ps = [ps_o(), ps_o()]
                pts = {}

                def emit_scores(jc):
                    s_ps = ps_s()
                    for e in range(2):
                        nc.tensor.matmul(
                            s_ps[:, 512 * e:512 * (e + 1)],
                            lhsT=k_res[t][64 * e:64 * (e + 1),
                                          128 * jc:128 * (jc + 1)],
                            rhs=q_res[t][64 * e:64 * (e + 1),
                                         IQ * iq:IQ * (iq + 1)],
                            start=True, stop=True,
                            tile_position=(64 * e, 0))
                    pt = attn.tile([128, 1024], F16, name="pt", tag="pt",
                                   bufs=6)
                    nc.scalar.activation(pt, s_ps, ACT_EXP, scale=INV_SCALE)
                    pts[jc] = pt

                def emit_av(jc):
                    pt = pts.pop(jc)
                    for e in range(2):
                        nc.tensor.matmul(
                            o_ps[e][0:65, :],
                            lhsT=vaug[t][:, jc, 65 * e:65 * (e + 1)],
                            rhs=pt[:, 512 * e:512 * (e + 1)],
                            start=(jc == 0), stop=(jc == NJC - 1),
                            skip_group_check=True)

                for jc in range(NJC):
                    emit_scores(jc)
                    if jc >= 2:
                        emit_av(jc - 2)
                    if jc >= 2 and fillers:
                        fillers.pop(0)()
                emit_av(NJC - 2)
                emit_av(NJC - 1)
                while fillers:
                    fillers.pop(0)()

                # normalization: denom rows -> DRAM -> reciprocal ->
                # DRAM -> partition-broadcast -> DVE multiply
                osb = []
                for e in range(2):
                    ot = attn.tile([65, IQ], F16, name="osb",
                                   tag=f"osb{e}", bufs=2)
                    nc.vector.tensor_copy(out=ot, in_=o_ps[e][0:65, :])
                    osb.append(ot)
                dnd = dram.tile([2, IQ], F16, name="dnd", tag=f"dnd{t}_{iq}")
                for e in range(2):
                    nc.gpsimd.dma_start(out=dnd[e:e + 1, :],
                                        in_=osb[e][64:65, :])
                dsc = attn.tile([128, 2, 4], F16, name="dsc", tag="dsc",
                                bufs=2)
                nc.sync.dma_start(
                    out=dsc,
                    in_=bass.AP(tensor=dnd.tensor, offset=dnd.offset,
                                ap=[[4, 128], [IQ, 2], [1, 4]]))
                rsc = attn.tile([128, 2, 4], F16, name="rsc", tag="rsc",
                                bufs=2)
                nc.vector.reciprocal(out=rsc, in_=dsc)
                rcd = dram.tile([128, 8], F16, name="rcd", tag=f"rcd{t}_{iq}")
                nc.gpsimd.dma_start(out=rcd, in_=rsc)
                for e in range(2):
                    rb = attn.tile([64, IQ], F16, name="rb", tag=f"rb{e}",
                                   bufs=2)
                    nc.sync.dma_start(
                        out=rb,
                        in_=bass.AP(tensor=rcd.tensor,
                                    offset=rcd.offset + 4 * e,
                                    ap=[[0, 64], [8, 128], [1, 4]]))
                    nc.vector.tensor_mul(
                        out=o_res[t][64 * e:64 * (e + 1),
                                     IQ * iq:IQ * (iq + 1)],
                        in0=osb[e][0:64, :], in1=rb)

            # ---------------- schedule ----------------
            for n in range(4):
                emit_qk_unit(0, n, "q")
            for n in range(4):
                emit_qk_unit(0, n, "k")
            for lc in range(6):
                emit_v_unit(lc)

            def vg(lc):
                return lambda: emit_v_unit(lc)

            def qkg(t, n, which):
                return lambda: emit_qk_unit(t, n, which)

            def og(iq, mi, n):
                return lambda: emit_o_unit(iq, mi, n)

            def qk_units(t):
                return ([qkg(t, n, "q") for n in range(4)]
                        + [qkg(t, n, "k") for n in range(4)])

            def o_units(iq):
                return [og(iq, mi, n) for mi in range(4) for n in range(2)]

            fillers_by_slot = {
                (0, 0): [vg(lc) for lc in range(6, NJC)],
                (0, 1): qk_units(1),
                (0, 3): qk_units(2),
                (1, 1): qk_units(3),
                (3, 1): o_units(0),
                (3, 2): o_units(1),
                (3, 3): o_units(2),
            }
            for t in range(NPAIR):
                for iq in range(NIQ):
                    emit_attn(t, iq, fillers_by_slot.get((t, iq), ()))
            for u in o_units(3):
                u()

    nc.compile()
    return nc


_NC_CACHE = []


def kernel_with_results(x, wq, wk, wv, wo, bo, **run_kwargs):
    x = np.asarray(x, dtype=np.float32)
    wqT = np.asarray(wq, dtype=np.float32).T.astype(np.float16)
    wkT = np.asarray(wk, dtype=np.float32).T.astype(np.float16)
    wvT = np.asarray(wv, dtype=np.float32).T.astype(np.float16)
    woT = np.asarray(wo, dtype=np.float32).T.astype(np.float16)
    bo = np.asarray(bo, dtype=np.float32)

    if not _NC_CACHE:
        _NC_CACHE.append(build_nc())
    nc = _NC_CACHE[0]

    in_maps = []
    for c in range(NCORES):
        b, hg = divmod(c, 2)
        xb = np.ascontiguousarray(x[b, :, 0, :]).astype(np.float16)
        sl = slice(NO * hg, NO * (hg + 1))
        in_maps.append({
            "x": xb,
            "wqT": np.ascontiguousarray(wqT[:, sl]),
            "wkT": np.ascontiguousarray(wkT[:, sl]),
            "wvT": np.ascontiguousarray(wvT[:, sl]),
            "woT": np.ascontiguousarray(woT[sl, :]),
            "ones16": np.ones((1, NJC), dtype=np.float16),
        })

    kres = run_bass_kernel_spmd(nc, in_maps, list(range(NCORES)), **run_kwargs)

    out = np.empty((B, D, 1, L), dtype=np.float32)
    for b in range(B):
        yT0 = kres.results[2 * b]["yT"].astype(np.float32)
        yT1 = kres.results[2 * b + 1]["yT"].astype(np.float32)
        out[b, :, 0, :] = (yT0 + yT1).T + bo[:, None]
    return out, kres


def kernel(x, wq, wk, wv, wo, bo):
    out, _ = kernel_with_results(x, wq, wk, wv, wo, bo)
    return out
